# revision 8
# baseline (speedup 1.0000x reference)
"""AdaptivePCEN Trainium2 kernel.

Data-parallel over batch: core i computes batches [4i, 4i+4) of the
[32, 128, 4000] input. PPN weights replicated. Per core:
  - PE (bf16): h = relu(W1^T [Xprev; X] + b1), gates = W2^T h + b2,
    laid out so each gate lands as a [F=128, T_chunk] PSUM tile.
  - ACT: sigmoid/softplus gate evacuations + exp/ln epilogue.
  - DVE: tensor_tensor_scan runs the EMA recurrence M_t = (1-s)M +
    s X along the free (time) axis in one instruction per batch.
"""

import numpy as np

B, F, T, H = 32, 128, 4000, 256
N_CORES = 8
BSH = B // N_CORES  # batches per core
CH = 500  # T-chunk; one PSUM bank per accumulation group (bank-crossing matmul output corrupts)
SUB = 500  # matmul free-dim per instruction (<=512 fp32-out psum bank)

_COMPILED = {}


def _build(bsh=BSH, t=T, ch=CH, sub=SUB):
    from contextlib import ExitStack

    import concourse.bass as bass
    import concourse.tile as tile
    from concourse import bacc, mybir
    from concourse.tile_rust import add_dep_helper

    f32 = mybir.dt.float32
    bf16 = mybir.dt.bfloat16
    AF = mybir.ActivationFunctionType
    OP = mybir.AluOpType
    EPS = 1e-6

    nc = bacc.Bacc(
        "TRN2",
        target_bir_lowering=False,
        debug=False,
        num_devices=N_CORES,
    )

    X = nc.dram_tensor("X", [bsh * F, t], f32, kind="ExternalInput").ap()
    W1 = nc.dram_tensor("W1", [2 * F, H], f32, kind="ExternalInput").ap()
    b1 = nc.dram_tensor("b1", [H, 1], f32, kind="ExternalInput").ap()
    W2 = nc.dram_tensor("W2", [H, 4 * F], f32, kind="ExternalInput").ap()
    b2 = nc.dram_tensor("b2", [4 * F, 1], f32, kind="ExternalInput").ap()
    out = nc.dram_tensor("out", [bsh * F, t], f32, kind="ExternalOutput").ap()

    nch = t // ch
    assert t % ch == 0 and ch % sub == 0

    with tile.TileContext(nc) as tc, ExitStack() as ctx:
        const = ctx.enter_context(tc.tile_pool(name="const", bufs=1))
        stag = ctx.enter_context(tc.tile_pool(name="stag", bufs=1))
        xpool = ctx.enter_context(tc.tile_pool(name="xpool", bufs=2))
        hpsum = ctx.enter_context(tc.tile_pool(name="hpsum", bufs=2, space="PSUM"))
        gpsum = ctx.enter_context(tc.tile_pool(name="gpsum", bufs=3, space="PSUM"))
        hsb = ctx.enter_context(tc.tile_pool(name="hsb", bufs=2))
        gates = ctx.enter_context(tc.tile_pool(name="gates", bufs=2))
        tmp = ctx.enter_context(tc.tile_pool(name="tmp", bufs=1))

        # ---- weights: DMA f32, cast to bf16 ----
        w1f = const.tile([F, 2 * H], f32, tag="w1f")
        nc.sync.dma_start(out=w1f[:, 0:H], in_=W1[0:F, :])
        nc.sync.dma_start(out=w1f[:, H : 2 * H], in_=W1[F : 2 * F, :])
        w1 = const.tile([F, 2 * H], bf16, tag="w1")
        nc.vector.tensor_copy(w1[:], w1f[:])
        w1a = w1[:, 0:H]  # W1 rows 0:F (Xprev part), [K=F, M=H]
        w1b = w1[:, H : 2 * H]  # W1 rows F:2F (X part)

        w2f = const.tile([F, 8 * F], f32, tag="w2f")
        nc.sync.dma_start(out=w2f[:, 0 : 4 * F], in_=W2[0:F, :])
        nc.sync.dma_start(out=w2f[:, 4 * F : 8 * F], in_=W2[F : 2 * F, :])
        w2 = const.tile([F, 8 * F], bf16, tag="w2")
        nc.vector.tensor_copy(w2[:], w2f[:])
        w2a = w2[:, 0 : 4 * F]  # W2 rows 0:H/2 (h1 part), [K, 4F]
        w2b = w2[:, 4 * F : 8 * F]  # W2 rows H/2:H (h2 part)

        bias1 = const.tile([F, 2], f32, tag="bias1")
        nc.sync.dma_start(out=bias1[:, 0:1], in_=b1[0:F, :])
        nc.sync.dma_start(out=bias1[:, 1:2], in_=b1[F : 2 * F, :])
        bias2 = const.tile([F, 4], f32, tag="bias2")
        for g in range(4):
            nc.sync.dma_start(
                out=bias2[:, g : g + 1], in_=b2[g * F : (g + 1) * F, :]
            )
        epsb = const.tile([F, 1], f32, tag="epsb")
        nc.vector.memset(epsb[:], EPS)

        prev_act = [None]  # last ACT instruction of previous batch
        first_act = [None]

        def act(out_ap, in_ap, func, **kw):
            inst = nc.scalar.activation(out_ap, in_ap, func, **kw)
            if first_act[0] is None:
                first_act[0] = inst
            return inst

        for b in range(bsh):
            # ---- load X[b] and cast to bf16 with 2-col lead layout ----
            # xbuf col j (j>=2) = X[b,:,j-2]; col 1 = X[b,:,0] (X_prev edge)
            # Xcur view = xbuf[:, 2:t+2] (4B aligned), Xprev = xbuf[:, 1:t+1]
            xf = stag.tile([F, t], f32, tag="xf")
            nc.sync.dma_start(out=xf[:], in_=X[b * F : (b + 1) * F, :])
            xbuf = xpool.tile([F, t + 4], bf16, tag="xbuf")
            nc.vector.tensor_copy(xbuf[:, 2 : t + 2], xf[:])
            nc.vector.tensor_copy(xbuf[:, 1:2], xf[:, 0:1])
            xcur = xbuf[:, 2 : t + 2]

            s_sb = gates.tile([F, t], bf16, tag="s")
            al_sb = gates.tile([F, t], bf16, tag="al")
            r_sb = gates.tile([F, t], bf16, tag="r")
            zd_sb = gates.tile([F, t], bf16, tag="zd")

            batch_first_act = [None]

            for c in range(nch):
                t0 = c * ch
                hp1 = hpsum.tile([F, ch], f32, tag="h1")
                hp2 = hpsum.tile([F, ch], f32, tag="h2")
                for s0 in range(0, ch, sub):
                    xp = xbuf[:, 1 + t0 + s0 : 1 + t0 + s0 + sub]
                    xc = xbuf[:, 2 + t0 + s0 : 2 + t0 + s0 + sub]
                    nc.tensor.matmul(
                        hp1[:, s0 : s0 + sub], w1a[:, 0:F], xp,
                        start=True, stop=False,
                    )
                    nc.tensor.matmul(
                        hp1[:, s0 : s0 + sub], w1b[:, 0:F], xc,
                        start=False, stop=True,
                    )
                    nc.tensor.matmul(
                        hp2[:, s0 : s0 + sub], w1a[:, F:H], xp,
                        start=True, stop=False,
                    )
                    nc.tensor.matmul(
                        hp2[:, s0 : s0 + sub], w1b[:, F:H], xc,
                        start=False, stop=True,
                    )
                h1s = hsb.tile([F, ch], bf16, tag="h1s")
                h2s = hsb.tile([F, ch], bf16, tag="h2s")
                nc.vector.tensor_scalar(
                    h1s[:], hp1[:], bias1[:, 0:1], 0.0, OP.add, OP.max
                )
                nc.vector.tensor_scalar(
                    h2s[:], hp2[:], bias1[:, 1:2], 0.0, OP.add, OP.max
                )

                # gates sequentially: s, alpha, r (ACT sigmoid), zd (DVE)
                for g, dest in ((0, s_sb), (1, al_sb), (3, r_sb), (2, zd_sb)):
                    gp = gpsum.tile([F, ch], f32, tag="g")
                    for s0 in range(0, ch, sub):
                        nc.tensor.matmul(
                            gp[:, s0 : s0 + sub],
                            w2a[:, g * F : (g + 1) * F], h1s[:, s0 : s0 + sub],
                            start=True, stop=False,
                        )
                        nc.tensor.matmul(
                            gp[:, s0 : s0 + sub],
                            w2b[:, g * F : (g + 1) * F], h2s[:, s0 : s0 + sub],
                            start=False, stop=True,
                        )
                    if g == 2:
                        nc.vector.tensor_scalar(
                            zd_sb[:, t0 : t0 + ch], gp[:], bias2[:, 2:3],
                            None, OP.add,
                        )
                    else:
                        inst = act(
                            dest[:, t0 : t0 + ch], gp[:], AF.Sigmoid,
                            bias=bias2[:, g : g + 1],
                        )
                        if batch_first_act[0] is None:
                            batch_first_act[0] = inst

            # keep ACT table sets grouped: this batch's sigmoids after the
            # previous batch's ln/exp epilogue
            if prev_act[0] is not None and batch_first_act[0] is not None:
                add_dep_helper(
                    batch_first_act[0].ins, prev_act[0].ins, sync=False,
                    reason="act table grouping",
                )

            # ---- phase B: softplus via exp/ln, scan, PCEN epilogue ----
            # (this toolchain has no Softplus LUT; ln(1+exp(z)) keeps all
            # phase-B ACT ops inside the natural_log_exp table set)
            ez = tmp.tile([F, t], bf16, tag="ez")
            act(ez[:], zd_sb[:], AF.Exp)
            dl_sb = tmp.tile([F, t], bf16, tag="dl")
            i_dl = act(dl_sb[:], ez[:], AF.Ln, bias=1.0)

            a_sb = tmp.tile([F, t], bf16, tag="a")
            nc.vector.tensor_scalar(a_sb[:], s_sb[:], -1.0, 1.0, OP.mult, OP.add)
            bb_sb = tmp.tile([F, t], bf16, tag="bb")
            nc.vector.tensor_tensor(bb_sb[:], s_sb[:], xcur, OP.mult)

            M = tmp.tile([F, t], f32, tag="A")
            nc.vector.tensor_tensor_scan(
                M[:], a_sb[:], bb_sb[:], 0.0, OP.mult, OP.add
            )

            L = tmp.tile([F, t], f32, tag="B")
            i_L = act(L[:], M[:], AF.Ln, bias=epsb[:])
            add_dep_helper(i_L.ins, i_dl.ins, sync=False, reason="sp before nl")

            t1 = tmp.tile([F, t], f32, tag="A")
            nc.vector.tensor_tensor(t1[:], al_sb[:], L[:], OP.mult)
            e1 = tmp.tile([F, t], bf16, tag="B")
            act(e1[:], t1[:], AF.Exp, scale=-1.0)
            num = tmp.tile([F, t], bf16, tag="A")
            nc.vector.tensor_tensor(num[:], xcur, e1[:], OP.mult)
            base = tmp.tile([F, t], bf16, tag="B")
            nc.vector.tensor_tensor(base[:], num[:], dl_sb[:], OP.add)
            lb = tmp.tile([F, t], bf16, tag="A")
            act(lb[:], base[:], AF.Ln)
            t2 = tmp.tile([F, t], bf16, tag="B")
            nc.vector.tensor_tensor(t2[:], r_sb[:], lb[:], OP.mult)
            p1 = tmp.tile([F, t], f32, tag="A")
            act(p1[:], t2[:], AF.Exp)

            ld = tmp.tile([F, t], bf16, tag="C")
            act(ld[:], dl_sb[:], AF.Ln)
            t3 = tmp.tile([F, t], bf16, tag="B")
            nc.vector.tensor_tensor(t3[:], r_sb[:], ld[:], OP.mult)
            p2 = tmp.tile([F, t], f32, tag="C")
            i_p2 = act(p2[:], t3[:], AF.Exp)
            prev_act[0] = i_p2

            ob = tmp.tile([F, t], f32, tag="ob")
            nc.vector.tensor_tensor(ob[:], p1[:], p2[:], OP.subtract)
            nc.sync.dma_start(out=out[b * F : (b + 1) * F, :], in_=ob[:])

    nc.compile()
    return nc


def _get(key=(BSH, T, CH, SUB)):
    if key not in _COMPILED:
        _COMPILED[key] = _build(*key)
    return _COMPILED[key]


def _in_maps(X, W1, b1, W2, b2):
    maps = []
    for i in range(N_CORES):
        maps.append(
            {
                "X": np.ascontiguousarray(
                    X[i * BSH : (i + 1) * BSH].reshape(BSH * F, T)
                ),
                "W1": np.ascontiguousarray(W1),
                "b1": np.ascontiguousarray(b1.reshape(H, 1)),
                "W2": np.ascontiguousarray(W2),
                "b2": np.ascontiguousarray(b2.reshape(4 * F, 1)),
            }
        )
    return maps


def run(X, W1, b1, W2, b2, trace=False, **kw):
    from concourse.bass_utils import run_bass_kernel_spmd

    nc = _get()
    res = run_bass_kernel_spmd(
        nc,
        _in_maps(X, W1, b1, W2, b2),
        core_ids=list(range(N_CORES)),
        trace=trace,
        **kw,
    )
    out = np.concatenate(
        [res.results[i]["out"].reshape(BSH, F, T) for i in range(N_CORES)],
        axis=0,
    ).astype(np.float32)
    return out, res


def kernel(X, W1, b1, W2, b2):
    return run(X, W1, b1, W2, b2)[0]
